# revision 3
# baseline (speedup 1.0000x reference)
"""Capsule-routing kernel for Trainium2 (8 NeuronCores, data-parallel over batch).

Math (u_hat is never materialized):
  u_hat[b,j,n,:] = u[b,n,:] @ W_j          (W_j = W[:, j*16:(j+1)*16])
  iter1: c uniform=0.1 -> o1[j] = 0.1*(sum_n u[n,:]) @ W_j   (host, tiny)
  iter t: Q[:,j] = W_j @ o[j];  logits b = u @ Q;  c = softmax_j(b)
          R.T[:,j] = sum_n c[n,j]*u[n,:]  (accumulated on PE)
  out = squash(R3 @ W_j)  (host epilogue, 64x160)

Per core: 8 samples. u is loaded ONCE per layout in fp16 (11-bit mantissa,
same as f32r): u.T chunks [f,n] are the logits stationaries, u chunks [n,f]
are the R stationaries; both movers are tiny ([128,10]).  Q is sent/produced
as fp16 hi+lo and the two logits matmuls accumulate hi+lo products in PSUM
(no DVE add).  Softmax runs batched per sample-group with per-n max
subtraction; exp is a single fp16 ACT op per group reading b from SBUF.
The o->Q chain runs on the PE (W_j / W_j.T stationaries, f32r), not DVE.
iter1's Q2 is precomputed on host from st = sum_n u.  Output is R3.T; the
final o3 = R3 @ W_j and squash run on host.
"""

import os
import sys

import numpy as np

for _p in ("/opt/trn_rl_repo", "/opt/trn_rl_repo/concourse"):
    if _p not in sys.path and os.path.isdir(_p):
        sys.path.insert(0, _p)

import concourse.bass as bass
import concourse.mybir as mybir
import concourse.tile as tile
from concourse import bacc

F32 = mybir.dt.float32
F32R = mybir.dt.float32r
F16 = mybir.dt.float16
AF = mybir.ActivationFunctionType
AX = mybir.AxisListType
ALU = mybir.AluOpType

N_CORES = 8
B_FULL, N, D = 64, 2048, 128
J, DC = 10, 16
JD = J * DC          # 160
NT = N // 128        # 16 chunks of n per sample
B_LOC = B_FULL // N_CORES  # 8 samples per core
EPS = 1e-7
GROUPS = [(0, 6), (6, 2)]  # (start, size): small tail group to shrink the
                           # post-DMA critical path
WARM = 12


def _bcast(ap, extra):
    """Append step-0 (broadcast) dims to an AP."""
    return bass.AP(tensor=ap.tensor, offset=ap.offset,
                   ap=list(ap.ap) + [[0, n] for n in extra])


def build_program():
    nc = bacc.Bacc(None)

    utT_d = nc.declare_dram_parameter("utT", [B_LOC, D, N], F16, isOutput=False)
    unm_d = nc.declare_dram_parameter("unm", [B_LOC, D, NT, D], F16,
                                      isOutput=False)
    q2i_d = nc.declare_dram_parameter("q2i", [D, B_LOC, 2, J], F16,
                                      isOutput=False)
    w_d = nc.declare_dram_parameter("w", [D, J, DC], F32R, isOutput=False)
    wt_d = nc.declare_dram_parameter("wt", [DC, J, D], F32R, isOutput=False)
    out_d = nc.declare_dram_parameter("out", [D, B_LOC, J], F32, isOutput=True)

    with tile.TileContext(nc) as tc:
        with (
            tc.tile_pool(name="consts", bufs=1) as consts,
            tc.tile_pool(name="big", bufs=1) as big,
            tc.tile_pool(name="sm", bufs=2) as sm,
            tc.tile_pool(name="chain", bufs=2) as chain,
            tc.tile_pool(name="pwarm", bufs=1, space="PSUM") as pwarm,
            tc.tile_pool(name="pbp", bufs=3, space="PSUM") as pbp,
            tc.tile_pool(name="prts", bufs=2, space="PSUM") as prts,
            tc.tile_pool(name="pch", bufs=1, space="PSUM") as pch,
        ):
            w_sb = consts.tile([D, J, DC], F32R)
            wt_sb = consts.tile([DC, J, D], F32R)
            q2i_sb = consts.tile([D, B_LOC, 2, J], F16)
            nc.sync.dma_start(out=w_sb[:], in_=w_d[:])
            nc.sync.dma_start(out=q2i_sb[:], in_=q2i_d[:])
            nc.scalar.dma_start(out=wt_sb[:], in_=wt_d[:])

            utT = [big.tile([D, NT, D], F16, tag=f"utT{s}", name=f"utT{s}")
                   for s in range(B_LOC)]
            unm = [big.tile([D, NT, D], F16, tag=f"unm{s}", name=f"unm{s}")
                   for s in range(B_LOC)]
            rings = [nc.sync, nc.scalar]
            for s in range(B_LOC):
                rings[s % 2].dma_start(
                    out=utT[s][:],
                    in_=utT_d[s, :, :].rearrange("p (t n) -> p t n", t=NT))
            for s in range(B_LOC):
                rings[s % 2].dma_start(out=unm[s][:], in_=unm_d[s])

            # PE p-state warmup under the DMA shadow.
            warm_ps = pwarm.tile([D, D], F32, tag="warm")
            for _ in range(WARM):
                nc.tensor.matmul(warm_ps[:], w_sb[:, 0:8, :], w_sb[:, 0:8, :],
                                 start=True, stop=True)

            def logits_sample(s, qhi, qlo):
                """16x (LDW u.T chunk + 2 MMs accumulating Qhi/Qlo products)."""
                bp = pbp.tile([D, NT, J], F32, tag="bp")
                for k in range(NT):
                    nc.tensor.matmul(bp[:, k, :], utT[s][:, k, :], qhi,
                                     start=True, stop=False)
                    nc.tensor.matmul(bp[:, k, :], utT[s][:, k, :], qlo,
                                     start=False, stop=True)
                return bp

            def softmax_group(g0, gsz, bps, it):
                b_g = sm.tile([D, gsz, NT, J], F32, tag=f"b{g0}")
                for i, bp in enumerate(bps):
                    nc.scalar.activation(b_g[:, i, :, :], bp[:], AF.Copy)
                negm = sm.tile([D, gsz, NT], F32, tag=f"m{g0}")
                nc.vector.reduce_max(negm[:], b_g[:], axis=AX.X, negate=True)
                bs_g = sm.tile([D, gsz, NT, J], F16, tag=f"s{g0}")
                nc.vector.tensor_add(bs_g[:], b_g[:], _bcast(negm[:], [J]))
                e_g = sm.tile([D, gsz, NT, J], F16, tag=f"e{g0}")
                nc.scalar.activation(e_g[:], bs_g[:], AF.Exp)
                z_g = sm.tile([D, gsz, NT], F32, tag=f"z{g0}")
                nc.vector.reduce_sum(z_g[:], e_g[:], axis=AX.X)
                zr_g = sm.tile([D, gsz, NT], F32, tag=f"r{g0}")
                nc.vector.reciprocal(zr_g[:], z_g[:])
                c_g = sm.tile([D, gsz, NT, J], F16, tag=f"c{g0}")
                nc.vector.tensor_mul(c_g[:], e_g[:], _bcast(zr_g[:], [J]))
                return c_g

            def r_group(g0, gsz, c_g, rts_ps):
                for i in range(gsz):
                    s = g0 + i
                    for k in range(NT):
                        nc.tensor.matmul(rts_ps[:, s, :], unm[s][:, k, :],
                                         c_g[:, i, k, :], start=(k == 0),
                                         stop=(k == NT - 1))

            def chain_group(g0, gsz, rts_ps):
                """R.T -> o -> Q -> q2 (fp16 hi/lo) for one group, on PE."""
                rts_sb = chain.tile([D, J, gsz], F32R, tag=f"rs{g0}")
                nc.scalar.activation(
                    rts_sb[:],
                    rts_ps[:, g0:g0 + gsz, :].rearrange("p s j -> p j s"),
                    AF.Copy)
                o_ps = pch.tile([DC, J, gsz], F32, tag="o_ps")
                for j in range(J):
                    nc.tensor.matmul(o_ps[:, j, :], w_sb[:, j, :],
                                     rts_sb[:, j, :], start=True, stop=True)
                o_sb = chain.tile([DC, J, gsz], F32R, tag=f"os{g0}")
                nc.scalar.activation(o_sb[:], o_ps[:], AF.Copy)
                q_ps = pch.tile([D, J, gsz], F32, tag="q_ps")
                for j in range(J):
                    nc.tensor.matmul(q_ps[:, j, :], wt_sb[:, j, :],
                                     o_sb[:, j, :], start=True, stop=True)
                q2_g = chain.tile([D, gsz, 2, J], F16, tag=f"q2{g0}")
                q_t = q_ps[:].rearrange("p j s -> p s j")
                nc.scalar.activation(q2_g[:, :, 0, :], q_t, AF.Copy)
                nc.vector.scalar_tensor_tensor(
                    out=q2_g[:, :, 1, :], in0=q_t, scalar=1.0,
                    in1=q2_g[:, :, 0, :], op0=ALU.mult, op1=ALU.subtract)
                return q2_g

            def drain_out(g0, gsz, rts_ps, ring):
                ob = chain.tile([D, gsz, J], F32, tag=f"ob{g0}")
                nc.scalar.activation(ob[:], rts_ps[:, g0:g0 + gsz, :], AF.Copy)
                ring.dma_start(out=out_d[:, g0:g0 + gsz, :], in_=ob[:])

            # ---- iteration 2 ----
            (a0, asz), (b0, bsz) = GROUPS
            rts2 = prts.tile([D, B_LOC, J], F32, tag="rts")
            bps_a = [logits_sample(a0 + i, q2i_sb[:, a0 + i, 0, :],
                                   q2i_sb[:, a0 + i, 1, :]) for i in range(asz)]
            c2a = softmax_group(a0, asz, bps_a, 2)
            bps_b = [logits_sample(b0 + i, q2i_sb[:, b0 + i, 0, :],
                                   q2i_sb[:, b0 + i, 1, :]) for i in range(bsz)]
            c2b = softmax_group(b0, bsz, bps_b, 2)
            r_group(a0, asz, c2a, rts2)
            q3a = chain_group(a0, asz, rts2)
            # iter-3 logits for group A while group B's iter-2 R waits on DMA
            bps_a3 = [logits_sample(a0 + i, q3a[:, i, 0, :], q3a[:, i, 1, :])
                      for i in range(asz)]
            c3a = softmax_group(a0, asz, bps_a3, 3)
            r_group(b0, bsz, c2b, rts2)
            q3b = chain_group(b0, bsz, rts2)
            # ---- iteration 3 ----
            rts3 = prts.tile([D, B_LOC, J], F32, tag="rts")
            r_group(a0, asz, c3a, rts3)
            drain_out(a0, asz, rts3, nc.sync)
            bps_b3 = [logits_sample(b0 + i, q3b[:, i, 0, :], q3b[:, i, 1, :])
                      for i in range(bsz)]
            c3b = softmax_group(b0, bsz, bps_b3, 3)
            r_group(b0, bsz, c3b, rts3)
            drain_out(b0, bsz, rts3, nc.scalar)

    nc.compile()
    return nc


def _f32r(x):
    xi = np.ascontiguousarray(x, np.float32).view(np.uint32).astype(np.int64)
    bias = ((xi >> 12) & 1) + (1 << 11) - 1
    return (((xi + bias) >> 12) << 12).astype(np.uint32).view(np.float32)


def _squash(o):
    s2 = (o ** 2).sum(-1, keepdims=True)
    return o * s2 / ((1.0 + s2) * np.sqrt(s2 + EPS))


_NC = None


def _get_nc():
    global _NC
    if _NC is None:
        _NC = build_program()
    return _NC


def run_sharded(u_vecs: np.ndarray, W: np.ndarray, **kw):
    """Shard over 8 cores, run, return (full_output, BassKernelResults)."""
    from concourse.bass_utils import run_bass_kernel_spmd

    u_vecs = np.ascontiguousarray(u_vecs, dtype=np.float32)
    W = np.ascontiguousarray(W, dtype=np.float32)
    assert u_vecs.shape == (B_FULL, N, D) and W.shape == (D, JD)

    nc = _get_nc()
    Wr = _f32r(W).reshape(D, J, DC)
    w_arr = np.ascontiguousarray(Wr)
    wt_arr = np.ascontiguousarray(Wr.transpose(2, 1, 0))

    in_maps = []
    for k in range(N_CORES):
        us = u_vecs[k * B_LOC:(k + 1) * B_LOC]          # [8, 2048, 128] f32
        u16 = us.astype(np.float16)
        utT = np.ascontiguousarray(u16.transpose(0, 2, 1))  # [8, 128f, 2048n]
        unm = np.ascontiguousarray(
            u16.reshape(B_LOC, NT, D, D).transpose(0, 2, 1, 3))  # [8,128n,16,128f]
        # host iter-1: o1 = 0.1*(sum_n u) @ W; Q2 = W_j @ o1, as fp16 hi/lo
        o1 = 0.1 * np.einsum('sf,fjd->sjd', us.sum(axis=1), Wr)
        q2 = np.einsum('fjd,sjd->fsj', Wr, o1).astype(np.float32)
        qhi = q2.astype(np.float16)
        qlo = (q2 - qhi.astype(np.float32)).astype(np.float16)
        q2i = np.ascontiguousarray(
            np.stack([qhi, qlo], axis=2).transpose(0, 1, 2, 3))  # [128,8,2,10]
        in_maps.append({
            "utT": utT, "unm": unm, "q2i": q2i, "w": w_arr, "wt": wt_arr,
        })
    res = run_bass_kernel_spmd(nc, in_maps, core_ids=list(range(N_CORES)), **kw)
    # out: [128 f, 8 s, 10 j] per core = R3.T
    r3 = np.concatenate(
        [res.results[k]["out"].transpose(1, 2, 0) for k in range(N_CORES)],
        axis=0)                                          # [64, 10, 128]
    o3 = np.einsum('sjf,fjd->sjd', r3.astype(np.float64),
                   W.reshape(D, J, DC).astype(np.float64))
    out = _squash(o3.astype(np.float32))
    return out.astype(np.float32), res


def kernel(u_vecs: np.ndarray, W: np.ndarray) -> np.ndarray:
    out, _ = run_sharded(u_vecs, W)
    return out
